# revision 3
# baseline (speedup 1.0000x reference)
"""ARMANet (2-layer ARMAConv K=1,T=1 + mean-pool + fc) on 8 TRN2 NeuronCores.

Sharding: nodes (and their incoming edges) are partitioned across the 8
cores by destination node; weights replicated; the g-tables (dinv-scaled
activations) are all-gathered between layers; a tiny [64,65] all-reduce
merges the pooled sums.

Self-contained: hardcodes shapes for N=100000, E=1600000, IN=HID=64,
OUT=32, NUM_GRAPHS=64.
"""
import os
import sys

sys.path.insert(0, '/opt/trn_rl_repo')

import numpy as np
import ml_dtypes

import concourse.bass as bass
import concourse.tile as tile
from concourse import bacc, mybir
from concourse.bass_utils import run_bass_kernel_spmd

N = 100000
E = 1600000
IN_DIM = 64
HID = 64
OUT = 32
NUM_GRAPHS = 64

P = 128
NCORES = 8
SLICE = 12544              # nodes per core (padded)
WPC = SLICE // P           # 98 windows per core
NPAD = SLICE * NCORES      # 100352
NQ = 4                     # src quarters (int16 index limit)
QSIZE = NPAD // NQ         # 25088
CHUNK = 2048               # gather call size
TPC = CHUNK // P           # tiles per call = 16
SENTINEL = 600.0           # one-hot miss value for padded slots

F32 = mybir.dt.float32
BF16 = mybir.dt.bfloat16
I16 = mybir.dt.int16

_cache = {}


def _install_trace_hook():
    """Register the NTFF profile hook so trace=True works under axon."""
    import types
    if 'antenv.axon_hooks' in sys.modules:
        return
    mod = types.ModuleType("antenv.axon_hooks")
    mod._hook = None
    mod.set_axon_ntff_profile_hook = lambda h: setattr(mod, '_hook', h)
    mod.get_axon_ntff_profile_hook = lambda: mod._hook
    sys.modules['antenv.axon_hooks'] = mod
    import antenv
    antenv.axon_hooks = mod
    try:
        from trn_agent_boot.trn_boot import _ntff_profile_via_ctypes
        mod.set_axon_ntff_profile_hook(
            _ntff_profile_via_ctypes('/opt/axon/libaxon_pjrt.so'))
    except Exception:
        pass



def _prep(x, edge_index, batch, dinv):
    """Host-side shard prep. Returns per-core arrays + static schedule."""
    row = np.asarray(edge_index[0], dtype=np.int64)
    col = np.asarray(edge_index[1], dtype=np.int64)

    c_of = col // SLICE
    w_of = (col % SLICE) // P
    d_rel = (col % P).astype(np.float32)
    q_of = row // QSIZE
    src_rel = (row % QSIZE).astype(np.int16)

    key = ((c_of * WPC + w_of) * NQ + q_of).astype(np.int64)
    counts = np.bincount(key, minlength=NCORES * WPC * NQ).reshape(NCORES, WPC, NQ)
    T = -(-counts.max(axis=0) // P)          # [WPC, NQ] tiles per group

    sec_tiles = T.sum(axis=0)                # per quarter
    sec_slots = sec_tiles * P
    sec_slots_pad = -(-sec_slots // CHUNK) * CHUNK
    sec_base = np.concatenate([[0], np.cumsum(sec_slots_pad)])[:-1]
    n_slots = int(sec_slots_pad.sum())
    n_tiles = n_slots // P
    n_calls = n_slots // CHUNK

    # group slot base (same for all cores): base[w, q]
    cumT = np.zeros((WPC, NQ), np.int64)
    for q in range(NQ):
        cumT[:, q] = np.concatenate([[0], np.cumsum(T[:, q])])[:-1]
    group_base = sec_base[None, :] + cumT * P   # [WPC, NQ]

    # per-edge slot assignment
    order = np.argsort(((c_of * NQ + q_of) * WPC + w_of), kind='stable')
    sorted_key = ((c_of * NQ + q_of) * WPC + w_of)[order]
    # rank within group
    change = np.empty(E, bool)
    change[0] = True
    change[1:] = sorted_key[1:] != sorted_key[:-1]
    starts = np.flatnonzero(change)
    grp_id = np.cumsum(change) - 1
    rank = np.arange(E) - starts[grp_id]
    slot = group_base[w_of[order], q_of[order]] + rank

    srcrel_slots = np.zeros((NCORES, n_slots), np.int16)
    drel_slots = np.full((NCORES, n_slots), SENTINEL, np.float32)
    cs = c_of[order]
    srcrel_slots[cs, slot] = src_rel[order]
    drel_slots[cs, slot] = d_rel[order]

    # call -> quarter
    q_of_call = np.zeros(n_calls, np.int64)
    for q in range(NQ):
        a = sec_base[q] // CHUNK
        b = (sec_base[q] + sec_slots_pad[q]) // CHUNK
        q_of_call[a:b] = q

    # tile metadata: (w, first, last) for real tiles, None for call-pad tiles
    tile_info = [None] * n_tiles
    for q in range(NQ):
        tb = sec_base[q] // P
        for w in range(WPC):
            t0 = tb + cumT[w, q]
            for j in range(T[w, q]):
                tile_info[t0 + j] = (w, q, j == 0, j == T[w, q] - 1)

    # flush role per (w): ordered list of quarters with T>0
    flush_q = [[q for q in range(NQ) if T[w, q] > 0] for w in range(WPC)]

    # idx int16 wrapped layout [128, n_slots//16]
    idx16 = np.zeros((NCORES, P, n_slots // 16), np.int16)
    for c in range(NCORES):
        bb = srcrel_slots[c].reshape(n_calls, P, 16)
        cols = np.concatenate([bb[k].T for k in range(n_calls)], axis=1)  # [16, ncalls*128]
        for g in range(8):
            idx16[c, g * 16:(g + 1) * 16, :] = cols

    dcol = np.zeros((NCORES, P, n_tiles), np.float32)
    for c in range(NCORES):
        dcol[c] = drel_slots[c].reshape(n_tiles, P).T

    # node-side per-core arrays
    xpad = np.zeros((NPAD, IN_DIM), np.float32)
    xpad[:N] = x
    dinv_pad = np.zeros(NPAD, np.float32)
    dinv_pad[:N] = dinv
    batch_pad = np.full(NPAD, SENTINEL, np.float32)
    batch_pad[:N] = np.asarray(batch, np.float32)

    xTa = np.ones((NCORES, IN_DIM + 1, SLICE), np.float32)
    dinv_cols = np.zeros((NCORES, P, WPC), np.float32)
    batch_cols = np.zeros((NCORES, P, WPC), np.float32)
    for c in range(NCORES):
        sl = slice(c * SLICE, (c + 1) * SLICE)
        xTa[c, :IN_DIM, :] = xpad[sl].T
        dinv_cols[c] = dinv_pad[sl].reshape(WPC, P).T
        batch_cols[c] = batch_pad[sl].reshape(WPC, P).T

    sched = dict(n_slots=n_slots, n_tiles=n_tiles, n_calls=n_calls,
                 q_of_call=q_of_call, tile_info=tile_info, flush_q=flush_q)
    data = dict(idx16=idx16, dcol=dcol, xTa=xTa, dinv_cols=dinv_cols,
                batch_cols=batch_cols)
    return sched, data


def _build(sched):
    """Build the SPMD Bass program (same for all cores)."""
    from contextlib import ExitStack

    n_calls = sched['n_calls']
    n_tiles = sched['n_tiles']
    q_of_call = sched['q_of_call']
    tile_info = sched['tile_info']
    flush_q = sched['flush_q']

    nc = bacc.Bacc("TRN2", target_bir_lowering=False, debug=False,
                   num_devices=NCORES, num_swdge_queues=4)

    # I/O
    xTa_d = nc.dram_tensor("xTa", [IN_DIM + 1, SLICE], F32, kind="ExternalInput")
    dinv_d = nc.dram_tensor("dinv_cols", [P, WPC], F32, kind="ExternalInput")
    batch_d = nc.dram_tensor("batch_cols", [P, WPC], F32, kind="ExternalInput")
    idx_d = nc.dram_tensor("idx16", [P, sched['n_slots'] // 16], I16, kind="ExternalInput")
    dcol_d = nc.dram_tensor("dcol", [P, n_tiles], F32, kind="ExternalInput")
    w1i_d = nc.dram_tensor("w1i", [IN_DIM, HID], F32, kind="ExternalInput")
    w1rb_d = nc.dram_tensor("w1rb", [IN_DIM + 1, HID], F32, kind="ExternalInput")
    w2i_d = nc.dram_tensor("w2i", [HID, HID], F32, kind="ExternalInput")
    w2rb_d = nc.dram_tensor("w2rb", [HID + 1, HID], F32, kind="ExternalInput")
    fcwb_d = nc.dram_tensor("fcwb", [HID + 1, OUT], F32, kind="ExternalInput")
    out_d = nc.dram_tensor("out", [NUM_GRAPHS, OUT], F32, kind="ExternalOutput")

    # constants embedded in NEFF
    iota_np = np.tile(np.arange(P, dtype=np.float32)[None, :], (P, 1))
    iota_d = nc.inline_tensor(iota_np, name="iota128")
    ident_d = nc.inline_tensor(np.eye(P, dtype=np.float32), name="ident128")

    # internal DRAM
    gloc = [nc.dram_tensor(f"gloc{l}", [SLICE, P], BF16) for l in (1, 2)]
    gfull = [nc.dram_tensor(f"gfull{l}", [NPAD, P], BF16, addr_space="Shared")
             for l in (1, 2)]
    pin_d = nc.dram_tensor("pooled_in", [NUM_GRAPHS, HID + 1], F32)
    pout_d = nc.dram_tensor("pooled_out", [NUM_GRAPHS, HID + 1], F32,
                            addr_space="Shared")

    RG = [list(range(NCORES))]

    with tile.TileContext(nc) as tc:
        with ExitStack() as ctx:
            consts = ctx.enter_context(tc.tile_pool(name="consts", bufs=1))
            canvas = ctx.enter_context(tc.tile_pool(name="canvas", bufs=1))
            gpool = ctx.enter_context(tc.tile_pool(name="gbuf", bufs=6))
            ohpool = ctx.enter_context(tc.tile_pool(name="oh", bufs=6))
            wk = ctx.enter_context(tc.tile_pool(name="wk", bufs=3))
            pp = ctx.enter_context(tc.tile_pool(name="pp", bufs=7, space="PSUM"))
            ppool = ctx.enter_context(tc.tile_pool(name="ppool", bufs=1, space="PSUM"))

            iota_sb = consts.tile([P, P], F32)
            nc.sync.dma_start(out=iota_sb[:], in_=iota_d[:, :])
            ident_sb = consts.tile([P, P], F32)
            nc.sync.dma_start(out=ident_sb[:], in_=ident_d[:, :])
            dinv_sb = consts.tile([P, WPC], F32)
            nc.sync.dma_start(out=dinv_sb[:], in_=dinv_d[:, :])
            batch_sb = consts.tile([P, WPC], F32)
            nc.sync.dma_start(out=batch_sb[:], in_=batch_d[:, :])
            w1i_sb = consts.tile([IN_DIM, HID], F32)
            nc.sync.dma_start(out=w1i_sb[:], in_=w1i_d[:, :])
            w1rb_sb = consts.tile([IN_DIM + 1, HID], F32)
            nc.sync.dma_start(out=w1rb_sb[:], in_=w1rb_d[:, :])
            w2i_sb = consts.tile([HID, HID], F32)
            nc.sync.dma_start(out=w2i_sb[:], in_=w2i_d[:, :])
            w2rb_sb = consts.tile([HID + 1, HID], F32)
            nc.sync.dma_start(out=w2rb_sb[:], in_=w2rb_d[:, :])
            fcwb_sb = consts.tile([HID + 1, OUT], F32)
            nc.sync.dma_start(out=fcwb_sb[:], in_=fcwb_d[:, :])
            idx_sb = consts.tile([P, sched['n_slots'] // 16], I16)
            nc.sync.dma_start(out=idx_sb[:], in_=idx_d[:, :])
            dcol_sb = consts.tile([P, n_tiles], F32)
            nc.sync.dma_start(out=dcol_sb[:], in_=dcol_d[:, :])
            xTa_sb = consts.tile([IN_DIM + 1, SLICE], F32)
            nc.sync.dma_start(out=xTa_sb[:], in_=xTa_d[:, :])

            S_c = canvas.tile([P, WPC * HID], F32)        # segment-sum canvas
            root2_c = canvas.tile([P, WPC * HID], F32)    # precomputed layer-2 root
            stage = canvas.tile([P, WPC * P], BF16)       # bf16 g staging (256B rows)
            nc.vector.memset(stage[:], 0.0)

            def stage_window(w, src_psum):
                """dinv-scale (or copy) PSUM z-window into bf16 staging."""
                nc.vector.tensor_scalar_mul(
                    stage[:, w * P: w * P + HID], src_psum[:], dinv_sb[:, w:w + 1])

            # ---- phase 0: g1 = dinv * (x @ w1_init), staged bf16 ----
            for w in range(WPC):
                zp = pp.tile([P, HID], F32, tag="ps")
                nc.tensor.matmul(zp[:], lhsT=xTa_sb[0:IN_DIM, w * P:(w + 1) * P],
                                 rhs=w1i_sb[:], start=True, stop=True)
                stage_window(w, zp)
            nc.sync.dma_start(
                out=gloc[0].ap().rearrange("(w p) d -> p w d", p=P),
                in_=stage[:].rearrange("p (w d) -> p w d", d=P))
            nc.gpsimd.collective_compute(
                "AllGather", mybir.AluOpType.bypass, replica_groups=RG,
                ins=[gloc[0].ap().opt()], outs=[gfull[0].ap().opt()])

            def propagate(gsrc):
                """Gather + one-hot matmul segment sum into S canvas."""
                cur = {}
                for k in range(n_calls):
                    q = int(q_of_call[k])
                    gbuf = gpool.tile([P, TPC * P], BF16)
                    nc.gpsimd.dma_gather(
                        out_ap=gbuf[:].rearrange("p (s d) -> p s d", d=P),
                        in_ap=gsrc[q * QSIZE:(q + 1) * QSIZE, :],
                        idxs_ap=idx_sb[:, k * (CHUNK // 16):(k + 1) * (CHUNK // 16)],
                        num_idxs=CHUNK, num_idxs_reg=CHUNK, elem_size=P,
                        single_packet=False, queue_num=k % 4)
                    for tt in range(TPC):
                        t = k * TPC + tt
                        info = tile_info[t]
                        if info is None:
                            continue
                        w, qq, first, last = info
                        oh = ohpool.tile([P, P], BF16)
                        nc.vector.tensor_scalar(
                            out=oh[:], in0=iota_sb[:],
                            scalar1=dcol_sb[:, t:t + 1], scalar2=None,
                            op0=mybir.AluOpType.is_equal)
                        if first:
                            seg_ps = pp.tile([P, HID], F32, tag="ps")
                            cur[(w, qq)] = seg_ps
                        ps = cur[(w, qq)]
                        nc.tensor.matmul(
                            ps[:], lhsT=oh[:],
                            rhs=gbuf[:].rearrange("p (s d) -> p s d", d=P)[:, tt, 0:HID],
                            start=first, stop=last)
                        if last:
                            ps = cur.pop((w, qq))
                            if qq == flush_q[w][0]:
                                nc.vector.tensor_copy(S_c[:, w * HID:(w + 1) * HID], ps[:])
                            else:
                                nc.vector.tensor_add(
                                    S_c[:, w * HID:(w + 1) * HID],
                                    S_c[:, w * HID:(w + 1) * HID], ps[:])
                for w in range(WPC):
                    if not flush_q[w]:
                        nc.vector.memset(S_c[:, w * HID:(w + 1) * HID], 0.0)

            # ---- layer 1 propagate + assembly ----
            propagate(gfull[0])
            for w in range(WPC):
                wc = slice(w * HID, (w + 1) * HID)
                rp = pp.tile([P, HID], F32, tag="ps")
                nc.tensor.matmul(rp[:], lhsT=xTa_sb[:, w * P:(w + 1) * P],
                                 rhs=w1rb_sb[:], start=True, stop=True)
                tmp = wk.tile([P, HID], F32, tag="tmp")
                nc.vector.tensor_scalar_mul(tmp[:], S_c[:, wc], dinv_sb[:, w:w + 1])
                nc.vector.tensor_add(tmp[:], tmp[:], rp[:])
                h1 = wk.tile([P, HID], F32, tag="h1")
                nc.scalar.activation(h1[:], tmp[:], mybir.ActivationFunctionType.Relu)
                hs1 = wk.tile([P, HID], F32, tag="hs1")
                nc.vector.tensor_scalar_mul(hs1[:], h1[:], dinv_sb[:, w:w + 1])
                # root2 = h1 @ w2_root + b2  (via transpose + K-extended matmul)
                tp = pp.tile([P, P], F32, tag="ps")
                nc.tensor.transpose(tp[0:HID, :], h1[:], ident_sb[:])
                h1T = wk.tile([HID + 1, P], F32, tag="h1T")
                nc.vector.tensor_copy(h1T[0:HID, :], tp[0:HID, :])
                nc.vector.memset(h1T[HID:HID + 1, :], 1.0)
                r2 = pp.tile([P, HID], F32, tag="ps")
                nc.tensor.matmul(r2[:], lhsT=h1T[:], rhs=w2rb_sb[:], start=True, stop=True)
                nc.vector.tensor_copy(root2_c[:, wc], r2[:])
                # g2 = hs1 @ w2_init  (dinv already folded into hs1)
                tp2 = pp.tile([P, P], F32, tag="ps")
                nc.tensor.transpose(tp2[0:HID, :], hs1[:], ident_sb[:])
                hs1T = wk.tile([HID, P], F32, tag="hs1T")
                nc.vector.tensor_copy(hs1T[:], tp2[0:HID, :])
                g2 = pp.tile([P, HID], F32, tag="ps")
                nc.tensor.matmul(g2[:], lhsT=hs1T[:], rhs=w2i_sb[:], start=True, stop=True)
                nc.vector.tensor_copy(stage[:, w * P:w * P + HID], g2[:])
            nc.sync.dma_start(
                out=gloc[1].ap().rearrange("(w p) d -> p w d", p=P),
                in_=stage[:].rearrange("p (w d) -> p w d", d=P))
            nc.gpsimd.collective_compute(
                "AllGather", mybir.AluOpType.bypass, replica_groups=RG,
                ins=[gloc[1].ap().opt()], outs=[gfull[1].ap().opt()])

            # ---- layer 2 propagate + assembly + pooling ----
            propagate(gfull[1])
            pool_ps = ppool.tile([NUM_GRAPHS, HID + 1], F32)
            for w in range(WPC):
                wc = slice(w * HID, (w + 1) * HID)
                tmp = wk.tile([P, HID], F32, tag="tmp")
                nc.vector.tensor_scalar_mul(tmp[:], S_c[:, wc], dinv_sb[:, w:w + 1])
                nc.vector.tensor_add(tmp[:], tmp[:], root2_c[:, wc])
                h2a = wk.tile([P, HID + 1], F32, tag="h2a")
                nc.scalar.activation(h2a[:, 0:HID], tmp[:],
                                     mybir.ActivationFunctionType.Relu)
                nc.vector.memset(h2a[:, HID:HID + 1], 1.0)
                ohB = wk.tile([P, NUM_GRAPHS], F32, tag="ohB")
                nc.vector.tensor_scalar(
                    out=ohB[:], in0=iota_sb[:, 0:NUM_GRAPHS],
                    scalar1=batch_sb[:, w:w + 1], scalar2=None,
                    op0=mybir.AluOpType.is_equal)
                nc.tensor.matmul(pool_ps[:], lhsT=ohB[:], rhs=h2a[:],
                                 start=(w == 0), stop=(w == WPC - 1))

            pooled_sb = wk.tile([NUM_GRAPHS, HID + 1], F32, tag="pooled")
            nc.vector.tensor_copy(pooled_sb[:], pool_ps[:])
            nc.sync.dma_start(out=pin_d[:, :], in_=pooled_sb[:])
            nc.gpsimd.collective_compute(
                "AllReduce", mybir.AluOpType.add, replica_groups=RG,
                ins=[pin_d.ap().opt()], outs=[pout_d.ap().opt()])
            pg = wk.tile([NUM_GRAPHS, HID + 1], F32, tag="pg")
            nc.sync.dma_start(out=pg[:], in_=pout_d[:, :])

            cnt = wk.tile([NUM_GRAPHS, 1], F32, tag="cnt")
            nc.vector.tensor_scalar_max(cnt[:], pg[:, HID:HID + 1], 1.0)
            nc.vector.reciprocal(cnt[:], cnt[:])
            pm = wk.tile([NUM_GRAPHS, HID], F32, tag="pm")
            nc.vector.tensor_scalar_mul(pm[:], pg[:, 0:HID], cnt[:])
            tpf = pp.tile([P, NUM_GRAPHS], F32, tag="ps")
            nc.tensor.transpose(tpf[0:HID, :], pm[:], ident_sb[0:NUM_GRAPHS, 0:NUM_GRAPHS])
            lhs_fc = wk.tile([HID + 1, NUM_GRAPHS], F32, tag="lhsfc")
            nc.vector.tensor_copy(lhs_fc[0:HID, :], tpf[0:HID, :])
            nc.vector.memset(lhs_fc[HID:HID + 1, :], 1.0)
            ops = pp.tile([NUM_GRAPHS, OUT], F32, tag="ps")
            nc.tensor.matmul(ops[:], lhsT=lhs_fc[:], rhs=fcwb_sb[:], start=True, stop=True)
            osb = wk.tile([NUM_GRAPHS, OUT], F32, tag="osb")
            nc.vector.tensor_copy(osb[:], ops[:])
            nc.sync.dma_start(out=out_d[:, :], in_=osb[:])

    nc.compile()
    return nc


def kernel(x, edge_index, batch, w1_init, w1_root, b1, w2_init, w2_root, b2,
           fc_w, fc_b):
    x = np.asarray(x, np.float32)
    edge_index = np.asarray(edge_index)
    batch = np.asarray(batch)

    col = edge_index[1].astype(np.int64)
    deg = np.bincount(col, minlength=N).astype(np.float32)
    dinv = np.where(deg > 0, 1.0 / np.sqrt(np.maximum(deg, 1.0)), 0.0).astype(np.float32)

    sched, data = _prep(x, edge_index, batch, dinv)

    key = (sched['n_slots'], sched['n_tiles'])
    if key not in _cache:
        _cache[key] = _build(sched)
    nc = _cache[key]

    w1rb = np.vstack([np.asarray(w1_root, np.float32), np.asarray(b1, np.float32)[None, :]])
    w2rb = np.vstack([np.asarray(w2_root, np.float32), np.asarray(b2, np.float32)[None, :]])
    fcwb = np.vstack([np.asarray(fc_w, np.float32), np.asarray(fc_b, np.float32)[None, :]])

    in_maps = []
    for c in range(NCORES):
        in_maps.append({
            "xTa": data['xTa'][c],
            "dinv_cols": data['dinv_cols'][c],
            "batch_cols": data['batch_cols'][c],
            "idx16": data['idx16'][c],
            "dcol": data['dcol'][c],
            "w1i": np.asarray(w1_init, np.float32),
            "w1rb": w1rb,
            "w2i": np.asarray(w2_init, np.float32),
            "w2rb": w2rb,
            "fcwb": fcwb,
        })

    trace = os.environ.get("GNN_TRACE", "0") == "1"
    kw = {}
    if trace:
        _install_trace_hook()
        kw = dict(trace=True, tmpdir=os.environ.get("GNN_TRACEDIR") or None)
    res = run_bass_kernel_spmd(nc, in_maps, core_ids=list(range(NCORES)), **kw)
    kernel.last_exec_time_ns = res.exec_time_ns
    return np.asarray(res.results[0]["out"], np.float32)
